# revision 20
# baseline (speedup 1.0000x reference)
"""CIN (Compressed Interaction Network) kernel for Trainium2, 8 NeuronCores.

Reference computation (per batch b, embedding dim d; x has 40 field vectors):
  h0[s] = relu( sum_{i,j} x_i x_j W0[i,j,s] + b0[s] )          s in 0..128
  nh    = h0[0:64];  d0 = h0[64:128]
  h1[s] = relu( sum_{i<40, j<64} x_i nh_j W1[i,j,s] + b1[s] )
  out   = concat(d0, h1, axis=s) summed over d                 -> (B, 192)

Strategy (v3)
-------------
Pure data parallel over the batch (B=2048 -> 256/core, 8192 bd columns,
processed in 8 chunks of 1024).  Per bd column both layers contract over
pairwise products; the two layers use different production schemes tuned
to engine capabilities:

  L0 (bf16, accuracy-critical: feeds d0 and nh): polarization.  PE
  projects 896 pair-sum features (x_i+x_j, K=40 one-hot sums), ACT
  squares them (PSUM->SBUF bf16), PE contracts (7 k-tiles, K=128,
  coefficients fold the singles corrections).
  L1 (fp8 tolerant: only d1): direct products.  Host-gathered broadcast
  tiles of x rows arrive from HBM in fp8 (tile m rows 0:64 = x_{2m},
  64:128 = x_{2m+1}); u2 = [nh; nh] built by one ACT relu + DVE copy;
  products x_i*nh_j on DVE/GpSimd straight to fp8; contraction = 10
  fp8 DoubleRow matmuls (2 k-tiles each, ~1.85x bf16).  Coefficients
  are scaled by 64 into fp8 range; the final relu un-scales.

d0/d1 relu outputs are bf16 so the 32-wide d-reduction on DVE runs in
2x mode.  Everything is software-pipelined two chunks deep.
"""

import numpy as np
import ml_dtypes

B, F0, D = 2048, 40, 32
NCORES = 8
BC = B // NCORES        # 256 batches per core
BD = BC * D             # 8192 bd columns per core
NH = 64                 # next-hidden fields (split_half of 128)
S0 = 128
S1 = 128

NFREE = 1024            # bd columns per pipeline chunk
NMM = 512               # max moving free dim per matmul instruction
NH2 = NFREE // NMM
NCHUNKS = BD // NFREE   # 8
NBPC = NFREE // D       # 32 batches per chunk

PAIRS0 = [(i, j) for i in range(F0) for j in range(i + 1, F0)]   # 780
NT0 = 7                 # L0 feature k-tiles (7*128 = 896 >= 820)
NT1 = 20                # L1 product k-tiles (20*128 = 2560)
C1SCALE = 64.0

GK2 = [2, 5, 8]         # k2 pairs whose products run on GpSimd (rest DVE)
DK2 = [k for k in range(NT1 // 2) if k not in GK2]
NDV = NT1 - 2 * len(GK2)

BF16 = ml_dtypes.bfloat16
FP8 = ml_dtypes.float8_e4m3fn

_cached = {}


def _l1_maps():
    """x-row index for L1 tile m, row p: i = 2m + p//64 (j = p%64)."""
    idx = np.zeros((NT1, 128), np.int64)
    for m in range(NT1):
        for p in range(128):
            idx[m, p] = 2 * m + p // 64
    return idx


_L1_IDX = _l1_maps()


def _build_host_weights(W0, b0, W1, b1):
    W0 = np.asarray(W0, np.float64)
    W1 = np.asarray(W1, np.float64)

    # ---- L0 polarization (pair-sum squares + singles corrections) ----
    p0 = np.zeros((F0, NT0 * 128), np.float64)
    c0 = np.zeros((NT0 * 128, S0), np.float64)
    Ssym = (W0 + W0.transpose(1, 0, 2)) / 2.0          # [i, j, s]
    for k, (i, j) in enumerate(PAIRS0):
        p0[i, k] = 1.0
        p0[j, k] = 1.0
        c0[k] = Ssym[i, j]
    for i in range(F0):
        k = len(PAIRS0) + i
        p0[i, k] = 1.0
        c0[k] = W0[i, i] - (Ssym[i].sum(axis=0) - Ssym[i, i])
    # c0 SBUF layout: feature f -> partition f%128, free col (f//128)*S0+s
    c0_sb = c0.reshape(NT0, 128, S0).transpose(1, 0, 2).reshape(128, NT0 * S0)

    # ---- L1 DoubleRow coefficients: [128, NT1//2, 2, S1] fp8 (x64) ----
    c1 = np.zeros((128, NT1 // 2, 2, S1), np.float64)
    for k2 in range(NT1 // 2):
        for tt in range(2):
            m = 2 * k2 + tt
            for p in range(128):
                c1[p, k2, tt] = W1[_L1_IDX[m, p], p % 64]
    c1 = np.clip(c1 * C1SCALE, -240, 240)

    return {
        "p0": p0.astype(BF16),
        "c0": c0_sb.astype(BF16),
        "c1": c1.reshape(128, (NT1 // 2) * 2 * S1).astype(FP8),
        "b0": np.asarray(b0, np.float32).reshape(S0, 1),
        "b1": np.asarray(b1, np.float32).reshape(S1, 1),
    }


def _split_multi_waits(nc):
    """The walrus build in this container rejects any instruction carrying
    more than one sync wait ("Too many sync wait commands").  Hoist all but
    one wait of every multi-wait instruction onto same-engine NOPs placed
    immediately before it (engines execute their stream in order, so this
    preserves the happens-before edges)."""
    import concourse.mybir as mybir

    n = 0
    for blk in nc.main_func.blocks:
        insts = blk.instructions
        out = []
        changed = False
        for inst in insts:
            si = getattr(inst, "sync_info", None)
            if si is not None and si.on_wait and len(si.on_wait) > 1:
                waits = list(si.on_wait)
                for w in waits[:-1]:
                    nop = mybir.InstNoOp(
                        name=f"waitsplit_{n}",
                        engine=inst.engine,
                        sync_info=mybir.SyncInfo(on_wait=[w], on_update=[]),
                        bass_nofuse=True,
                    )
                    n += 1
                    out.append(nop)
                si.on_wait = waits[-1:]
                changed = True
            out.append(inst)
        if changed:
            blk.instructions = out
    return n


def _build_nc():
    import concourse.bass as bass
    import concourse.tile as tile
    import concourse.mybir as mybir

    dt = mybir.dt
    AF = mybir.ActivationFunctionType
    ALU = mybir.AluOpType
    DR = mybir.MatmulPerfMode.DoubleRow

    nc = bass.Bass()

    xp_d = nc.dram_tensor("xp", [NCHUNKS, F0, NFREE], dt.bfloat16,
                          kind="ExternalInput")
    xb1_d = nc.dram_tensor("xb1", [NCHUNKS, 128, NDV * NFREE], dt.float8e4,
                           kind="ExternalInput")
    xb1g_d = nc.dram_tensor("xb1g", [NCHUNKS, 128, 2 * len(GK2) * NFREE],
                            dt.float8e4, kind="ExternalInput")
    p0_d = nc.dram_tensor("p0", [F0, NT0 * 128], dt.bfloat16,
                          kind="ExternalInput")
    c0_d = nc.dram_tensor("c0", [128, NT0 * S0], dt.bfloat16,
                          kind="ExternalInput")
    c1_d = nc.dram_tensor("c1", [128, (NT1 // 2) * 2 * S1], dt.float8e4,
                          kind="ExternalInput")
    b0_d = nc.dram_tensor("b0", [S0, 1], dt.float32, kind="ExternalInput")
    b1_d = nc.dram_tensor("b1", [S1, 1], dt.float32, kind="ExternalInput")
    out_d = nc.dram_tensor("out", [S0 - NH + S1, BC], dt.float32,
                           kind="ExternalOutput")

    with tile.TileContext(nc) as tc:
        with (
            tc.tile_pool(name="const", bufs=1) as const_pool,
            tc.tile_pool(name="xp", bufs=3) as xp_pool,
            tc.tile_pool(name="xb1", bufs=4) as xb1_pool,
            tc.tile_pool(name="sq", bufs=2) as sq_pool,
            tc.tile_pool(name="p1", bufs=2) as p1_pool,
            tc.tile_pool(name="u2", bufs=2) as u2_pool,
            tc.tile_pool(name="dd", bufs=2) as d_pool,
            tc.tile_pool(name="outp", bufs=1) as out_pool,
            tc.tile_pool(name="vps", bufs=2, space="PSUM") as vps_pool,
            tc.tile_pool(name="h0ps", bufs=1, space="PSUM") as h0_pool,
            tc.tile_pool(name="h1ps", bufs=1, space="PSUM") as h1_pool,
        ):
            p0_sb = const_pool.tile([F0, NT0 * 128], dt.bfloat16)
            c0_sb = const_pool.tile([128, NT0 * S0], dt.bfloat16)
            c1_sb = const_pool.tile([128, NT1 // 2, 2, S1], dt.float8e4)
            b0_sb = const_pool.tile([S0, 1], dt.float32)
            b1_sb = const_pool.tile([S1, 1], dt.float32)
            nc.gpsimd.dma_start(out=p0_sb[:], in_=p0_d[:])
            nc.gpsimd.dma_start(out=c0_sb[:], in_=c0_d[:])
            nc.gpsimd.dma_start(out=c1_sb[:], in_=c1_d[:])
            nc.gpsimd.dma_start(out=b0_sb[:], in_=b0_d[:])
            nc.gpsimd.dma_start(out=b1_sb[:], in_=b1_d[:])

            out0_sb = out_pool.tile([S0 - NH, BC], dt.float32, tag="o0")
            out1_sb = out_pool.tile([S1, BC], dt.float32, tag="o1")

            st = {}

            def dma_in(c):
                st[c] = {
                    "xp": xp_pool.tile([F0, NFREE], dt.bfloat16, tag="xp",
                                       name=f"xp_{c}"),
                    "xb1": xb1_pool.tile([128, NDV, NFREE], dt.float8e4,
                                         tag="xb1", name=f"xb1_{c}"),
                    "xb1g": xb1_pool.tile([128, 2 * len(GK2), NFREE],
                                          dt.float8e4, tag="xb1g",
                                          name=f"xb1g_{c}"),
                }
                nc.sync.dma_start(out=st[c]["xp"][:], in_=xp_d[c])
                nc.sync.dma_start(out=st[c]["xb1"][:], in_=xb1_d[c])
                nc.sync.dma_start(out=st[c]["xb1g"][:], in_=xb1g_d[c])

            def proj_sq(c):
                """L0: project pair-sums (PE) + square (ACT) per k-tile."""
                s = st[c]
                s["sq"] = sq_pool.tile([128, NT0, NFREE], dt.bfloat16,
                                       tag="sq", name=f"sq_{c}")
                for t in range(NT0):
                    vps = vps_pool.tile([128, NFREE], dt.float32, tag="vps",
                                        name=f"vps_{c}_{t}")
                    for h in range(NH2):
                        hs = slice(h * NMM, (h + 1) * NMM)
                        mi = nc.tensor.matmul(
                            vps[:, hs], p0_sb[:, t * 128:(t + 1) * 128],
                            s["xp"][:, hs], start=True, stop=True,
                        )
                        if h > 0:
                            mi.ins.ldweights = False
                    nc.scalar.square(s["sq"][:, t, :], vps[:])

            def l0_contract(c):
                s = st[c]
                s["h0ps"] = h0_pool.tile([S0, NFREE], dt.float32, tag="h0",
                                         name=f"h0_{c}")
                for t in range(NT0):
                    for h in range(NH2):
                        hs = slice(h * NMM, (h + 1) * NMM)
                        mi = nc.tensor.matmul(
                            s["h0ps"][:, hs], c0_sb[:, t * S0:(t + 1) * S0],
                            s["sq"][:, t, hs], start=(t == 0),
                            stop=(t == NT0 - 1),
                        )
                        if h > 0:
                            mi.ins.ldweights = False

            def post0(c):
                s = st[c]
                s["u2"] = u2_pool.tile([128, NFREE], dt.bfloat16, tag="u2",
                                       name=f"u2_{c}")
                nc.scalar.activation(s["u2"][0:NH, :], s["h0ps"][0:NH, :],
                                     AF.Relu, bias=b0_sb[0:NH, 0:1], scale=1.0)
                # second [nh] copy + a private full copy for GpSimd (avoids
                # SBUF contention with DVE reads), via the scalar HWDGE ring
                # (the sync ring is FIFO-ordered behind the bulk xb1 loads).
                nc.scalar.dma_start(out=s["u2"][NH:128, :],
                                    in_=s["u2"][0:NH, :])
                s["u2g"] = u2_pool.tile([128, NFREE], dt.bfloat16, tag="u2g",
                                        name=f"u2g_{c}")
                nc.scalar.dma_start(out=s["u2g"][:], in_=s["u2"][:])
                d0 = d_pool.tile([S0 - NH, NBPC, D], dt.bfloat16, tag="d0")
                nc.scalar.activation(d0[:], s["h0ps"][NH:S0, :], AF.Relu,
                                     bias=b0_sb[NH:S0, 0:1], scale=1.0)
                nc.vector.tensor_reduce(
                    out=out0_sb[:, c * NBPC:(c + 1) * NBPC], in_=d0[:],
                    axis=mybir.AxisListType.X, op=ALU.add,
                )

            def prod1(c):
                # one 2048-col op per k2 pair: in0 = u2 broadcast along the
                # pair dim (stride 0), in1 = two adjacent fp8 x-tiles.
                # bf16 operand must come FIRST (fp8 in0 is a 2.6x slow path).
                s = st[c]
                s["prod1"] = p1_pool.tile([128, NDV // 2, 2, NFREE],
                                          dt.float8e4, tag="prod1",
                                          name=f"prod1_{c}")
                s["prod1g"] = p1_pool.tile([128, len(GK2), 2, NFREE],
                                           dt.float8e4, tag="prod1g",
                                           name=f"prod1g_{c}")
                u2b = s["u2"][:].rearrange("p (o n) -> p o n", o=1) \
                    .broadcast_to([128, 2, NFREE])
                for k2 in range(NT1 // 2):
                    if k2 in GK2:
                        # GpSimd: plain ops on fully private tiles (broadcast
                        # APs and SBUF sharing with DVE are both slow paths)
                        gi = GK2.index(k2)
                        for tt in range(2):
                            nc.gpsimd.tensor_mul(
                                s["prod1g"][:, gi, tt, :],
                                s["u2g"][:], s["xb1g"][:, 2 * gi + tt, :])
                    else:
                        dvi = DK2.index(k2)
                        nc.vector.tensor_mul(
                            s["prod1"][:, dvi, :, :], u2b,
                            s["xb1"][:, 2 * dvi:2 * dvi + 2, :])

            def l1_contract(c):
                s = st[c]
                s["h1ps"] = h1_pool.tile([S1, NFREE], dt.float32, tag="h1",
                                         name=f"h1_{c}")
                HB = NMM // 2
                for h in range(NFREE // HB):
                    for k2 in range(NT1 // 2):
                        if k2 in GK2:
                            rhs = s["prod1g"][:, GK2.index(k2), :,
                                              h * HB:(h + 1) * HB]
                        else:
                            rhs = s["prod1"][:, DK2.index(k2), :,
                                             h * HB:(h + 1) * HB]
                        nc.tensor.matmul(
                            s["h1ps"][:, h * HB:(h + 1) * HB],
                            c1_sb[:, k2], rhs,
                            start=(k2 == 0), stop=(k2 == NT1 // 2 - 1),
                            perf_mode=DR,
                        )

            def post1(c):
                s = st[c]
                d1 = d_pool.tile([S1, NBPC, D], dt.bfloat16, tag="d1")
                nc.scalar.activation(d1[:], s["h1ps"][:], AF.Relu,
                                     bias=b1_sb[:, 0:1], scale=1.0 / C1SCALE)
                nc.vector.tensor_reduce(
                    out=out1_sb[:, c * NBPC:(c + 1) * NBPC], in_=d1[:],
                    axis=mybir.AxisListType.X, op=ALU.add,
                )
                del st[c]

            dma_in(0)
            dma_in(1)
            for i in range(NCHUNKS + 2):
                if i + 2 < NCHUNKS:
                    dma_in(i + 2)
                if i < NCHUNKS:
                    proj_sq(i)
                if 0 <= i - 1 < NCHUNKS:
                    l0_contract(i - 1)
                    post0(i - 1)
                    prod1(i - 1)
                if 0 <= i - 2 < NCHUNKS:
                    l1_contract(i - 2)
                    post1(i - 2)

            nc.gpsimd.dma_start(out=out_d[0:S0 - NH, :], in_=out0_sb[:])
            nc.gpsimd.dma_start(out=out_d[S0 - NH:, :], in_=out1_sb[:])

    _split_multi_waits(nc)
    return nc


def kernel(x, W0, b0, W1, b1):
    from concourse.bass_utils import run_bass_kernel_spmd

    x = np.asarray(x)
    w = _build_host_weights(W0, b0, W1, b1)

    if "nc" not in _cached:
        _cached["nc"] = _build_nc()
    nc = _cached["nc"]

    in_maps = []
    for c in range(NCORES):
        xs = x[c * BC:(c + 1) * BC]                       # [256, 40, 32]
        xc = np.ascontiguousarray(
            xs.transpose(1, 0, 2).reshape(F0, BD)).astype(np.float32)
        xp = np.ascontiguousarray(
            xc.astype(BF16).reshape(F0, NCHUNKS, NFREE).transpose(1, 0, 2))
        xc8 = np.clip(xc, -240, 240).astype(FP8).reshape(F0, NCHUNKS, NFREE)
        dv_tiles = [m_ for k2 in DK2 for m_ in (2 * k2, 2 * k2 + 1)]
        gp_tiles = [m_ for k2 in GK2 for m_ in (2 * k2, 2 * k2 + 1)]
        xb1 = xc8[_L1_IDX[dv_tiles]]                      # [NDV,128,NC,NFREE]
        xb1 = np.ascontiguousarray(xb1.transpose(2, 1, 0, 3).reshape(
            NCHUNKS, 128, NDV * NFREE))
        xb1g = xc8[_L1_IDX[gp_tiles]]
        xb1g = np.ascontiguousarray(xb1g.transpose(2, 1, 0, 3).reshape(
            NCHUNKS, 128, len(gp_tiles) * NFREE))
        m = {"xp": xp, "xb1": xb1, "xb1g": xb1g}
        m.update(w)
        in_maps.append(m)

    import os
    trace = bool(os.environ.get("CIN_TRACE"))
    res = run_bass_kernel_spmd(nc, in_maps, list(range(NCORES)), trace=trace)
    _cached["last_res"] = res
    outs = []
    for c in range(NCORES):
        o = res.results[c]["out"]                         # [192, 256]
        outs.append(np.ascontiguousarray(o.T))            # [256, 192]
    return np.concatenate(outs, axis=0).astype(np.float32)


# revision 21
# speedup vs baseline: 1.1212x; 1.1212x over previous
"""CIN (Compressed Interaction Network) kernel for Trainium2, 8 NeuronCores.

Reference computation (per batch b, embedding dim d; x has 40 field vectors):
  h0[s] = relu( sum_{i,j} x_i x_j W0[i,j,s] + b0[s] )          s in 0..128
  nh    = h0[0:64];  d0 = h0[64:128]
  h1[s] = relu( sum_{i<40, j<64} x_i nh_j W1[i,j,s] + b1[s] )
  out   = concat(d0, h1, axis=s) summed over d                 -> (B, 192)

Strategy (v3)
-------------
Pure data parallel over the batch (B=2048 -> 256/core, 8192 bd columns,
processed in 8 chunks of 1024).  Per bd column both layers contract over
pairwise products; the two layers use different production schemes tuned
to engine capabilities:

  L0 (bf16, accuracy-critical: feeds d0 and nh): polarization.  PE
  projects 896 pair-sum features (x_i+x_j, K=40 one-hot sums), ACT
  squares them (PSUM->SBUF bf16), PE contracts (7 k-tiles, K=128,
  coefficients fold the singles corrections).
  L1 (fp8 tolerant: only d1): direct products.  Host-gathered broadcast
  tiles of x rows arrive from HBM in fp8 (tile m rows 0:64 = x_{2m},
  64:128 = x_{2m+1}); u2 = [nh; nh] built by one ACT relu + DVE copy;
  products x_i*nh_j on DVE/GpSimd straight to fp8; contraction = 10
  fp8 DoubleRow matmuls (2 k-tiles each, ~1.85x bf16).  Coefficients
  are scaled by 64 into fp8 range; the final relu un-scales.

d0/d1 relu outputs are bf16 so the 32-wide d-reduction on DVE runs in
2x mode.  Everything is software-pipelined two chunks deep.
"""

import numpy as np
import ml_dtypes

B, F0, D = 2048, 40, 32
NCORES = 8
BC = B // NCORES        # 256 batches per core
BD = BC * D             # 8192 bd columns per core
NH = 64                 # next-hidden fields (split_half of 128)
S0 = 128
S1 = 128

NFREE = 1024            # bd columns per pipeline chunk
NMM = 512               # max moving free dim per matmul instruction
NH2 = NFREE // NMM
NCHUNKS = BD // NFREE   # 8
NBPC = NFREE // D       # 32 batches per chunk

PAIRS0 = [(i, j) for i in range(F0) for j in range(i + 1, F0)]   # 780
NT0 = 7                 # L0 feature k-tiles (7*128 = 896 >= 820)
NT1 = 20                # L1 product k-tiles (20*128 = 2560)
C1SCALE = 64.0

GK2 = []                # k2 pairs whose products run on GpSimd (rest DVE)
DK2 = [k for k in range(NT1 // 2) if k not in GK2]
NDV = NT1 - 2 * len(GK2)

BF16 = ml_dtypes.bfloat16
FP8 = ml_dtypes.float8_e4m3fn

_cached = {}


def _l1_maps():
    """x-row index for L1 tile m, row p: i = 2m + p//64 (j = p%64)."""
    idx = np.zeros((NT1, 128), np.int64)
    for m in range(NT1):
        for p in range(128):
            idx[m, p] = 2 * m + p // 64
    return idx


_L1_IDX = _l1_maps()


def _build_host_weights(W0, b0, W1, b1):
    W0 = np.asarray(W0, np.float64)
    W1 = np.asarray(W1, np.float64)

    # ---- L0 polarization (pair-sum squares + singles corrections) ----
    p0 = np.zeros((F0, NT0 * 128), np.float64)
    c0 = np.zeros((NT0 * 128, S0), np.float64)
    Ssym = (W0 + W0.transpose(1, 0, 2)) / 2.0          # [i, j, s]
    for k, (i, j) in enumerate(PAIRS0):
        p0[i, k] = 1.0
        p0[j, k] = 1.0
        c0[k] = Ssym[i, j]
    for i in range(F0):
        k = len(PAIRS0) + i
        p0[i, k] = 1.0
        c0[k] = W0[i, i] - (Ssym[i].sum(axis=0) - Ssym[i, i])
    # c0 SBUF layout: feature f -> partition f%128, free col (f//128)*S0+s
    c0_sb = c0.reshape(NT0, 128, S0).transpose(1, 0, 2).reshape(128, NT0 * S0)

    # ---- L1 DoubleRow coefficients: [128, NT1//2, 2, S1] fp8 (x64) ----
    c1 = np.zeros((128, NT1 // 2, 2, S1), np.float64)
    for k2 in range(NT1 // 2):
        for tt in range(2):
            m = 2 * k2 + tt
            for p in range(128):
                c1[p, k2, tt] = W1[_L1_IDX[m, p], p % 64]
    c1 = np.clip(c1 * C1SCALE, -240, 240)

    return {
        "p0": p0.astype(BF16),
        "c0": c0_sb.astype(BF16),
        "c1": c1.reshape(128, (NT1 // 2) * 2 * S1).astype(FP8),
        "b0": np.asarray(b0, np.float32).reshape(S0, 1),
        "b1": np.asarray(b1, np.float32).reshape(S1, 1),
    }


def _split_multi_waits(nc):
    """The walrus build in this container rejects any instruction carrying
    more than one sync wait ("Too many sync wait commands").  Hoist all but
    one wait of every multi-wait instruction onto same-engine NOPs placed
    immediately before it (engines execute their stream in order, so this
    preserves the happens-before edges)."""
    import concourse.mybir as mybir

    n = 0
    for blk in nc.main_func.blocks:
        insts = blk.instructions
        out = []
        changed = False
        for inst in insts:
            si = getattr(inst, "sync_info", None)
            if si is not None and si.on_wait and len(si.on_wait) > 1:
                waits = list(si.on_wait)
                for w in waits[:-1]:
                    nop = mybir.InstNoOp(
                        name=f"waitsplit_{n}",
                        engine=inst.engine,
                        sync_info=mybir.SyncInfo(on_wait=[w], on_update=[]),
                        bass_nofuse=True,
                    )
                    n += 1
                    out.append(nop)
                si.on_wait = waits[-1:]
                changed = True
            out.append(inst)
        if changed:
            blk.instructions = out
    return n


def _build_nc():
    import concourse.bass as bass
    import concourse.tile as tile
    import concourse.mybir as mybir

    dt = mybir.dt
    AF = mybir.ActivationFunctionType
    ALU = mybir.AluOpType
    DR = mybir.MatmulPerfMode.DoubleRow

    nc = bass.Bass()

    xp_d = nc.dram_tensor("xp", [NCHUNKS, F0, NFREE], dt.bfloat16,
                          kind="ExternalInput")
    xb1_d = nc.dram_tensor("xb1", [NCHUNKS, 128, NDV * NFREE], dt.float8e4,
                           kind="ExternalInput")
    xb1g_d = (nc.dram_tensor("xb1g", [NCHUNKS, 128, 2 * len(GK2) * NFREE],
                             dt.float8e4, kind="ExternalInput")
              if GK2 else None)
    p0_d = nc.dram_tensor("p0", [F0, NT0 * 128], dt.bfloat16,
                          kind="ExternalInput")
    c0_d = nc.dram_tensor("c0", [128, NT0 * S0], dt.bfloat16,
                          kind="ExternalInput")
    c1_d = nc.dram_tensor("c1", [128, (NT1 // 2) * 2 * S1], dt.float8e4,
                          kind="ExternalInput")
    b0_d = nc.dram_tensor("b0", [S0, 1], dt.float32, kind="ExternalInput")
    b1_d = nc.dram_tensor("b1", [S1, 1], dt.float32, kind="ExternalInput")
    out_d = nc.dram_tensor("out", [S0 - NH + S1, BC], dt.float32,
                           kind="ExternalOutput")

    with tile.TileContext(nc) as tc:
        with (
            tc.tile_pool(name="const", bufs=1) as const_pool,
            tc.tile_pool(name="xp", bufs=3) as xp_pool,
            tc.tile_pool(name="xb1", bufs=4) as xb1_pool,
            tc.tile_pool(name="sq", bufs=2) as sq_pool,
            tc.tile_pool(name="p1", bufs=2) as p1_pool,
            tc.tile_pool(name="u2", bufs=2) as u2_pool,
            tc.tile_pool(name="dd", bufs=2) as d_pool,
            tc.tile_pool(name="outp", bufs=1) as out_pool,
            tc.tile_pool(name="vps", bufs=2, space="PSUM") as vps_pool,
            tc.tile_pool(name="h0ps", bufs=1, space="PSUM") as h0_pool,
            tc.tile_pool(name="h1ps", bufs=1, space="PSUM") as h1_pool,
        ):
            p0_sb = const_pool.tile([F0, NT0 * 128], dt.bfloat16)
            c0_sb = const_pool.tile([128, NT0 * S0], dt.bfloat16)
            c1_sb = const_pool.tile([128, NT1 // 2, 2, S1], dt.float8e4)
            b0_sb = const_pool.tile([S0, 1], dt.float32)
            b1_sb = const_pool.tile([S1, 1], dt.float32)
            nc.gpsimd.dma_start(out=p0_sb[:], in_=p0_d[:])
            nc.gpsimd.dma_start(out=c0_sb[:], in_=c0_d[:])
            nc.gpsimd.dma_start(out=c1_sb[:], in_=c1_d[:])
            nc.gpsimd.dma_start(out=b0_sb[:], in_=b0_d[:])
            nc.gpsimd.dma_start(out=b1_sb[:], in_=b1_d[:])

            out0_sb = out_pool.tile([S0 - NH, BC], dt.float32, tag="o0")
            out1_sb = out_pool.tile([S1, BC], dt.float32, tag="o1")

            st = {}

            def dma_in(c):
                st[c] = {
                    "xp": xp_pool.tile([F0, NFREE], dt.bfloat16, tag="xp",
                                       name=f"xp_{c}"),
                    "xb1": xb1_pool.tile([128, NDV, NFREE], dt.float8e4,
                                         tag="xb1", name=f"xb1_{c}"),
                }
                if GK2:
                    st[c]["xb1g"] = xb1_pool.tile(
                        [128, 2 * len(GK2), NFREE], dt.float8e4, tag="xb1g",
                        name=f"xb1g_{c}")
                    nc.sync.dma_start(out=st[c]["xb1g"][:], in_=xb1g_d[c])
                nc.sync.dma_start(out=st[c]["xp"][:], in_=xp_d[c])
                nc.sync.dma_start(out=st[c]["xb1"][:], in_=xb1_d[c])

            def proj_sq(c):
                """L0: project pair-sums (PE) + square (ACT) per k-tile."""
                s = st[c]
                s["sq"] = sq_pool.tile([128, NT0, NFREE], dt.bfloat16,
                                       tag="sq", name=f"sq_{c}")
                for t in range(NT0):
                    vps = vps_pool.tile([128, NFREE], dt.float32, tag="vps",
                                        name=f"vps_{c}_{t}")
                    for h in range(NH2):
                        hs = slice(h * NMM, (h + 1) * NMM)
                        mi = nc.tensor.matmul(
                            vps[:, hs], p0_sb[:, t * 128:(t + 1) * 128],
                            s["xp"][:, hs], start=True, stop=True,
                        )
                        if h > 0:
                            mi.ins.ldweights = False
                    nc.scalar.square(s["sq"][:, t, :], vps[:])

            def l0_contract(c):
                s = st[c]
                s["h0ps"] = h0_pool.tile([S0, NFREE], dt.float32, tag="h0",
                                         name=f"h0_{c}")
                for t in range(NT0):
                    for h in range(NH2):
                        hs = slice(h * NMM, (h + 1) * NMM)
                        mi = nc.tensor.matmul(
                            s["h0ps"][:, hs], c0_sb[:, t * S0:(t + 1) * S0],
                            s["sq"][:, t, hs], start=(t == 0),
                            stop=(t == NT0 - 1),
                        )
                        if h > 0:
                            mi.ins.ldweights = False

            def post0(c):
                s = st[c]
                s["u2"] = u2_pool.tile([128, NFREE], dt.bfloat16, tag="u2",
                                       name=f"u2_{c}")
                nc.scalar.activation(s["u2"][0:NH, :], s["h0ps"][0:NH, :],
                                     AF.Relu, bias=b0_sb[0:NH, 0:1], scale=1.0)
                # second [nh] copy + a private full copy for GpSimd (avoids
                # SBUF contention with DVE reads), via the scalar HWDGE ring
                # (the sync ring is FIFO-ordered behind the bulk xb1 loads).
                nc.scalar.dma_start(out=s["u2"][NH:128, :],
                                    in_=s["u2"][0:NH, :])
                if GK2:
                    s["u2g"] = u2_pool.tile([128, NFREE], dt.bfloat16,
                                            tag="u2g", name=f"u2g_{c}")
                    nc.scalar.dma_start(out=s["u2g"][:], in_=s["u2"][:])
                d0 = d_pool.tile([S0 - NH, NBPC, D], dt.bfloat16, tag="d0")
                nc.scalar.activation(d0[:], s["h0ps"][NH:S0, :], AF.Relu,
                                     bias=b0_sb[NH:S0, 0:1], scale=1.0)
                nc.vector.tensor_reduce(
                    out=out0_sb[:, c * NBPC:(c + 1) * NBPC], in_=d0[:],
                    axis=mybir.AxisListType.X, op=ALU.add,
                )

            def prod1(c):
                # one 2048-col op per k2 pair: in0 = u2 broadcast along the
                # pair dim (stride 0), in1 = two adjacent fp8 x-tiles.
                # bf16 operand must come FIRST (fp8 in0 is a 2.6x slow path).
                s = st[c]
                s["prod1"] = p1_pool.tile([128, NDV // 2, 2, NFREE],
                                          dt.float8e4, tag="prod1",
                                          name=f"prod1_{c}")
                if GK2:
                    s["prod1g"] = p1_pool.tile([128, len(GK2), 2, NFREE],
                                               dt.float8e4, tag="prod1g",
                                               name=f"prod1g_{c}")
                u2b = s["u2"][:].rearrange("p (o n) -> p o n", o=1) \
                    .broadcast_to([128, 2, NFREE])
                for k2 in range(NT1 // 2):
                    if k2 in GK2:
                        # GpSimd: plain ops on fully private tiles (broadcast
                        # APs and SBUF sharing with DVE are both slow paths)
                        gi = GK2.index(k2)
                        for tt in range(2):
                            nc.gpsimd.tensor_mul(
                                s["prod1g"][:, gi, tt, :],
                                s["u2g"][:], s["xb1g"][:, 2 * gi + tt, :])
                    else:
                        dvi = DK2.index(k2)
                        nc.vector.tensor_mul(
                            s["prod1"][:, dvi, :, :], u2b,
                            s["xb1"][:, 2 * dvi:2 * dvi + 2, :])

            def l1_contract(c):
                s = st[c]
                s["h1ps"] = h1_pool.tile([S1, NFREE], dt.float32, tag="h1",
                                         name=f"h1_{c}")
                HB = NMM // 2
                for h in range(NFREE // HB):
                    for k2 in range(NT1 // 2):
                        if k2 in GK2:
                            rhs = s["prod1g"][:, GK2.index(k2), :,
                                              h * HB:(h + 1) * HB]
                        else:
                            rhs = s["prod1"][:, DK2.index(k2), :,
                                             h * HB:(h + 1) * HB]
                        nc.tensor.matmul(
                            s["h1ps"][:, h * HB:(h + 1) * HB],
                            c1_sb[:, k2], rhs,
                            start=(k2 == 0), stop=(k2 == NT1 // 2 - 1),
                            perf_mode=DR,
                        )

            def post1(c):
                s = st[c]
                d1 = d_pool.tile([S1, NBPC, D], dt.bfloat16, tag="d1")
                nc.scalar.activation(d1[:], s["h1ps"][:], AF.Relu,
                                     bias=b1_sb[:, 0:1], scale=1.0 / C1SCALE)
                nc.vector.tensor_reduce(
                    out=out1_sb[:, c * NBPC:(c + 1) * NBPC], in_=d1[:],
                    axis=mybir.AxisListType.X, op=ALU.add,
                )
                del st[c]

            dma_in(0)
            dma_in(1)
            for i in range(NCHUNKS + 2):
                if i + 2 < NCHUNKS:
                    dma_in(i + 2)
                if i < NCHUNKS:
                    proj_sq(i)
                if 0 <= i - 1 < NCHUNKS:
                    l0_contract(i - 1)
                    post0(i - 1)
                    prod1(i - 1)
                if 0 <= i - 2 < NCHUNKS:
                    l1_contract(i - 2)
                    post1(i - 2)

            nc.gpsimd.dma_start(out=out_d[0:S0 - NH, :], in_=out0_sb[:])
            nc.gpsimd.dma_start(out=out_d[S0 - NH:, :], in_=out1_sb[:])

    _split_multi_waits(nc)
    return nc


def kernel(x, W0, b0, W1, b1):
    from concourse.bass_utils import run_bass_kernel_spmd

    x = np.asarray(x)
    w = _build_host_weights(W0, b0, W1, b1)

    if "nc" not in _cached:
        _cached["nc"] = _build_nc()
    nc = _cached["nc"]

    in_maps = []
    for c in range(NCORES):
        xs = x[c * BC:(c + 1) * BC]                       # [256, 40, 32]
        xc = np.ascontiguousarray(
            xs.transpose(1, 0, 2).reshape(F0, BD)).astype(np.float32)
        xp = np.ascontiguousarray(
            xc.astype(BF16).reshape(F0, NCHUNKS, NFREE).transpose(1, 0, 2))
        xc8 = np.clip(xc, -240, 240).astype(FP8).reshape(F0, NCHUNKS, NFREE)
        dv_tiles = [m_ for k2 in DK2 for m_ in (2 * k2, 2 * k2 + 1)]
        gp_tiles = [m_ for k2 in GK2 for m_ in (2 * k2, 2 * k2 + 1)]
        xb1 = xc8[_L1_IDX[dv_tiles]]                      # [NDV,128,NC,NFREE]
        xb1 = np.ascontiguousarray(xb1.transpose(2, 1, 0, 3).reshape(
            NCHUNKS, 128, NDV * NFREE))
        m = {"xp": xp, "xb1": xb1}
        if gp_tiles:
            xb1g = xc8[_L1_IDX[gp_tiles]]
            m["xb1g"] = np.ascontiguousarray(
                xb1g.transpose(2, 1, 0, 3).reshape(
                    NCHUNKS, 128, len(gp_tiles) * NFREE))
        m.update(w)
        in_maps.append(m)

    import os
    trace = bool(os.environ.get("CIN_TRACE"))
    res = run_bass_kernel_spmd(nc, in_maps, list(range(NCORES)), trace=trace)
    _cached["last_res"] = res
    outs = []
    for c in range(NCORES):
        o = res.results[c]["out"]                         # [192, 256]
        outs.append(np.ascontiguousarray(o.T))            # [256, 192]
    return np.concatenate(outs, axis=0).astype(np.float32)
